# revision 1
# baseline (speedup 1.0000x reference)
"""Trainium2 Bass kernel for nn_ODEFunc_interaction (gnn_message_passing).

Math (see reference):
  dz_dt = tanh([z, t] @ vW1 + vb1) @ vW2 + vb2                    (v-net, all rows)
  for each pair (perm[2i], perm[2i+1]):
      d_i  = z[perm[2i]] - z[perm[2i+1]]
      g_i  = grad_phi(d_i) = pW1 @ (pW2[:,0] * (1 - tanh(d_i@pW1 + pb1)^2))
      out[perm[2i]]   = dz_dt[perm[2i]]   - g_i
      out[perm[2i+1]] = dz_dt[perm[2i+1]] + g_i
  last 3 rows (triple) handled on host in float64 (tiny).

Strategy: host gathers z[perm] so each of 8 cores owns a contiguous block of
200000/8 = 25000 rows (12500 pairs). On-device layout is transposed+packed:
X[128, 6250] where partition 32*j+d holds dim d of row-chunk j (4 chunks of
6250 rows). All matmuls run as fp32r (full-rate fp32) on PE sub-tiles via
tile_position quadrants; tanh (+bias) on ACT; pair-diff and square on GPSIMD;
(1-u^2) and final +/- combine on DVE. Host scatters the result back by perm.
"""

import os
import numpy as np

B, D, H = 200003, 32, 128
NCORES = 8
P2 = 200000            # rows covered by pairs
RPC = P2 // NCORES     # 25000 rows per core
NCHUNK = 4
L = RPC // NCHUNK      # 6250 packed columns per core
LP = L + 2             # padded to keep every fp32r matmul free-size even
G = 1024               # column block (2 PSUM banks)

_CACHE = {}
LAST_RESULTS = None    # BassKernelResults of the most recent run (for test.py)


def build_program():
    """Build the single-core Bass/Tile program (same program runs SPMD on 8 cores)."""
    from contextlib import ExitStack
    import concourse.bacc as bacc
    import concourse.mybir as mybir
    import concourse.tile as tile

    dt = mybir.dt
    F32, F32R = dt.float32, dt.float32r
    AF = mybir.ActivationFunctionType
    OP = mybir.AluOpType

    F16 = dt.float16
    # All matmul streams run in fp16 (fp32r measured ~3 cyc/col on HW; fp16
    # streams at 1 cyc/col and halves the input DMA). Accuracy ~4e-4 rel.
    # One concatenated fp16 weight tensor [128, 1536]:
    #   w1rep[0:128] | pw1rep[128:256] | w2q[256:768] | pwtq[768:1280]
    #   | w1z[1280:1408] | pw1z[1408:1536]
    # w2q/pwtq are column-placed per chunk (vW2 at columns 32j of block j,
    # zeros elsewhere): matmul outputs must start at PSUM partition 0, so the
    # 4 chunk matmuls accumulate full-M into one [128,*] psum tile.
    # w1z/pw1z: chunk 3 is read from partition base 64 with K=64 and zeros in
    # rows 64:96 (partition base 96 is not encodable).
    nc = bacc.Bacc()
    X = nc.dram_tensor("x", [128, LP], F16, kind="ExternalInput")
    WC = nc.dram_tensor("wcat", [128, 2048], F16, kind="ExternalInput")
    BC = nc.dram_tensor("bias", [128, 2], F32, kind="ExternalInput")
    O = nc.dram_tensor("out", [128, LP], F32, kind="ExternalOutput")

    with tile.TileContext(nc) as tc, ExitStack() as ctx:
        wpool = ctx.enter_context(tc.tile_pool(name="wpool", bufs=1))
        xpool = ctx.enter_context(tc.tile_pool(name="xpool", bufs=4))
        upool = ctx.enter_context(tc.tile_pool(name="upool", bufs=4))
        vpool = ctx.enter_context(tc.tile_pool(name="vpool", bufs=3))
        sqpool = ctx.enter_context(tc.tile_pool(name="sqpool", bufs=3))
        dpool = ctx.enter_context(tc.tile_pool(name="dpool", bufs=2))
        qspool = ctx.enter_context(tc.tile_pool(name="qspool", bufs=2))
        opool = ctx.enter_context(tc.tile_pool(name="opool", bufs=3))
        hps = ctx.enter_context(tc.tile_pool(name="hps", bufs=2, space="PSUM"))
        dzps = ctx.enter_context(tc.tile_pool(name="dzps", bufs=1, space="PSUM"))
        aps = ctx.enter_context(tc.tile_pool(name="aps", bufs=1, space="PSUM"))
        qps = ctx.enter_context(tc.tile_pool(name="qps", bufs=1, space="PSUM"))

        wt = wpool.tile([128, 2048], F16)
        nc.sync.dma_start(wt[:], WC[:])
        bt = wpool.tile([128, 2], F32)
        nc.sync.dma_start(bt[:], BC[:])
        w1 = wt[:, 0:128]
        pw1 = wt[:, 128:256]
        w2q = wt[:, 256:768]
        pwtq = wt[:, 768:1280]      # +pW1*w2 column-placed per chunk
        pwtqn = wt[:, 1280:1792]    # negated copy (odd output columns)
        w1z = wt[:, 1792:1920]
        pw1z = wt[:, 1920:2048]
        bh = bt[:, 0:1]
        pb1 = bt[:, 1:2]

        for c0 in range(0, LP, G):
            W_ = min(G, LP - c0)
            Wp = W_ // 2
            xt = xpool.tile([128, G], F16)
            nc.sync.dma_start(xt[:, :W_], X[:, c0 : c0 + W_])

            df = dpool.tile([128, G // 2], F16)
            nc.gpsimd.tensor_tensor(df[:, :Wp], xt[:, 0:W_:2], xt[:, 1:W_:2], OP.subtract)

            dz = dzps.tile([128, G], F32)
            qp = qps.tile([128, G // 2], F32)
            ot = opool.tile([128, G], F32)

            # j=3 first: its M=64 writes (start=True) clear psum partitions
            # 64:96 to zero; j=2 then accumulates its strip on top (start=False).
            for j in (3, 0, 1, 2):
                p0 = 32 * j
                ph = hps.tile([128, G], F32)
                for s0 in range(0, W_, 512):
                    sw = min(512, W_ - s0)
                    if j == 3:
                        nc.tensor.matmul(
                            ph[:, s0 : s0 + sw],
                            w1z[64:128],
                            xt[64:128, s0 : s0 + sw],
                            start=True,
                            stop=True,
                        )
                    else:
                        nc.tensor.matmul(
                            ph[:, s0 : s0 + sw],
                            w1[p0 : p0 + 32, :],
                            xt[p0 : p0 + 32, s0 : s0 + sw],
                            start=True,
                            stop=True,
                        )
                ut = upool.tile([128, G], F16)
                nc.scalar.activation(ut[:, :W_], ph[:, :W_], AF.Tanh, bias=bh[:])
                for s0 in range(0, W_, 512):
                    sw = min(512, W_ - s0)
                    nc.tensor.matmul(
                        dz[:, s0 : s0 + sw],
                        w2q[:, H * j : H * (j + 1)],
                        ut[:, s0 : s0 + sw],
                        start=(j == 3),
                        stop=(j == 2),
                        skip_group_check=True,
                    )
                pa = aps.tile([128, G // 2], F32)
                if j == 3:
                    nc.tensor.matmul(
                        pa[:, :Wp],
                        pw1z[64:128],
                        df[64:128, :Wp],
                        start=True,
                        stop=True,
                    )
                else:
                    nc.tensor.matmul(
                        pa[:, :Wp],
                        pw1[p0 : p0 + 32, :],
                        df[p0 : p0 + 32, :Wp],
                        start=True,
                        stop=True,
                    )
                vt = vpool.tile([128, G // 2], F16)
                nc.scalar.activation(vt[:, :Wp], pa[:, :Wp], AF.Tanh, bias=pb1[:])
                sq = sqpool.tile([128, G // 2], F16)
                nc.vector.tensor_mul(sq[:, :Wp], vt[:, :Wp], vt[:, :Wp])
                # q = pwtq^T v^2 accumulated over the 4 chunks; the constant
                # part of g = pwtq^T (1 - v^2) is folded on the host.
                nc.tensor.matmul(
                    qp[:, :Wp],
                    pwtq[:, H * j : H * (j + 1)],
                    sq[:, :Wp],
                    start=(j == 3),
                    stop=(j == 2),
                    skip_group_check=True,
                )

            qs = qspool.tile([128, G // 2], F32)
            nc.vector.tensor_copy(qs[:, :Wp], qp[:, :Wp])
            nc.vector.tensor_tensor(ot[:, 0:W_:2], dz[:, 0:W_:2], qs[:, :Wp], OP.add)
            nc.vector.tensor_tensor(ot[:, 1:W_:2], dz[:, 1:W_:2], qs[:, :Wp], OP.subtract)
            nc.sync.dma_start(O[:, c0 : c0 + W_], ot[:, :W_])

    nc.compile()
    return nc


def _prep_weights(t, vW1, vb1, vW2, vb2, pW1, pb1, pW2):
    f32 = np.float32
    t = np.asarray(t, dtype=f32).reshape(-1)[0]
    vW1 = np.asarray(vW1, dtype=f32)
    w1rep = np.tile(np.ascontiguousarray(vW1[:32]), (4, 1))            # [128,128]
    biash = (np.asarray(vb1, f32) + t * vW1[32]).reshape(128, 1).astype(f32)
    vw2 = np.ascontiguousarray(np.asarray(vW2, f32))                   # [128,32]
    pW1 = np.asarray(pW1, f32)
    pw1rep = np.tile(pW1, (4, 1))                                      # [128,128]
    pb1c = np.asarray(pb1, f32).reshape(128, 1).copy()
    w2col = np.asarray(pW2, f32).reshape(128)
    pw1tw2 = np.ascontiguousarray((pW1 * w2col[None, :]).T)            # [128,32]
    z96 = np.zeros((96, 128), f32)
    w2q = np.zeros((128, 512), f32)
    pwtq = np.zeros((128, 512), f32)
    for j in range(4):
        w2q[:, 128 * j + 32 * j : 128 * j + 32 * j + 32] = vw2
        pwtq[:, 128 * j + 32 * j : 128 * j + 32 * j + 32] = pw1tw2
    w1z = np.vstack([z96, vW1[:32]])                                   # [128,128]
    pw1z = np.vstack([z96, pW1])                                       # [128,128]
    wcat = np.hstack([w1rep, pw1rep, w2q, pwtq, -pwtq, w1z, pw1z]).astype(np.float16)
    bias = np.hstack([biash, pb1c]).astype(f32)
    # constant part of g: c0[d] = sum_k pW1[d,k]*w2[k], in the fp16 weight
    # precision actually used on device
    c0base = pw1tw2.astype(np.float16).astype(f32).sum(axis=0)         # [32]
    return {"wcat": np.ascontiguousarray(wcat), "bias": np.ascontiguousarray(bias),
            "_c0base": c0base}


def _pack_core(zc):
    """[25000, 32] f32 -> [128, 6252] fp16 packed (partition 32*j+d, col i =
    row j*L+i), padded with 2 zero columns."""
    out = np.zeros((128, LP), dtype=np.float16)
    out[:, :L] = zc.reshape(NCHUNK, L, 32).transpose(0, 2, 1).reshape(128, L)
    return out


def _unpack_core(oc):
    """[128, 6252] packed -> [25000, 32]."""
    return oc[:, :L].reshape(NCHUNK, 32, L).transpose(0, 2, 1).reshape(RPC, 32)


def _host_triple(t, z3, vW1, vb1, vW2, vb2, pW1, pb1, pW2):
    """Exact float64 computation of the 3 leftover rows: dz_dt + triple forces."""
    f8 = np.float64
    z3 = z3.astype(f8)
    vW1 = np.asarray(vW1, f8)
    t = float(np.asarray(t).reshape(-1)[0])
    h3 = np.tanh(z3 @ vW1[:32] + t * vW1[32] + np.asarray(vb1, f8))
    dz3 = h3 @ np.asarray(vW2, f8) + np.asarray(vb2, f8)

    pW1 = np.asarray(pW1, f8)
    w2 = np.asarray(pW2, f8).reshape(128)
    d9 = (z3[:, None, :] - z3[None, :, :]).reshape(9, 32)
    u9 = np.tanh(d9 @ pW1 + np.asarray(pb1, f8))
    s9 = (1.0 - u9 * u9) * w2[None, :]
    g9 = s9 @ pW1.T                       # grad_phi rows
    f9 = (-g9).reshape(3, 3, 32)
    f9 = f9 * (1.0 - np.eye(3)[:, :, None])
    force3 = f9.sum(axis=1) * 2.0
    return (dz3 + force3).astype(np.float32)


def kernel(t, z, perm, vW1, vb1, vW2, vb2, pW1, pb1, pW2, pb2):
    from concourse.bass_utils import run_bass_kernel_spmd

    global LAST_RESULTS
    if "nc" not in _CACHE:
        _CACHE["nc"] = build_program()
    nc = _CACHE["nc"]

    z = np.asarray(z, np.float32)
    perm = np.asarray(perm)
    weights = _prep_weights(t, vW1, vb1, vW2, vb2, pW1, pb1, pW2)

    c0base = weights.pop("_c0base")
    zg = z[perm[:P2]]                       # [200000, 32] gathered pair rows
    in_maps = []
    for c in range(NCORES):
        im = {"x": _pack_core(zg[c * RPC : (c + 1) * RPC])}
        im.update(weights)
        in_maps.append(im)

    trace = bool(int(os.environ.get("KERNEL_TRACE", "0")))
    res = run_bass_kernel_spmd(nc, in_maps, list(range(NCORES)), trace=trace)
    LAST_RESULTS = res

    out = np.empty((B, 32), dtype=np.float32)
    og = np.concatenate([_unpack_core(res.results[c]["out"]) for c in range(NCORES)], axis=0)
    vb2f = np.asarray(vb2, np.float32)
    og[0::2] += (vb2f - c0base)[None, :]
    og[1::2] += (vb2f + c0base)[None, :]
    out[perm[:P2]] = og
    out[perm[P2:]] = _host_triple(t, z[perm[P2:]], vW1, vb1, vW2, vb2, pW1, pb1, pW2)
    return out



# revision 7
# speedup vs baseline: 1.1037x; 1.1037x over previous
"""Trainium2 Bass kernel for nn_ODEFunc_interaction (gnn_message_passing).

Math (see reference):
  dz_dt = tanh([z, t] @ vW1 + vb1) @ vW2 + vb2                    (v-net, all rows)
  for each pair (perm[2i], perm[2i+1]):
      d_i  = z[perm[2i]] - z[perm[2i+1]]
      g_i  = grad_phi(d_i) = pW1 @ (pW2[:,0] * (1 - tanh(d_i@pW1 + pb1)^2))
      out[perm[2i]]   = dz_dt[perm[2i]]   - g_i
      out[perm[2i+1]] = dz_dt[perm[2i+1]] + g_i
  last 3 rows (triple) handled on host in float64 (tiny).

Strategy: host gathers z[perm] so each of 8 cores owns 25000 rows (12500
pairs).  On-device layout is transposed+packed: X[128, 6252] fp16 where
partition 32*j+d holds dim d of row-chunk j (4 chunks of 6250 rows + 1 pad
pair each).  Within each chunk, columns are grouped in blocks of 256 pairs
with the even pair-members in the first half of the block and the odd
members in the second half, so the pair-difference and the final +/-
combine all run on contiguous ranges (DVE 2x/4x fp16 modes, no strided
access).  Every matmul PSUM output starts at a bank boundary (mid-bank
starts are fatal on HW).  Per block: 4 h-matmuls -> fused tanh per 2
chunks -> 4 dz-matmuls accumulate; 4 pa-matmuls (2 banks, strided-AP
fused tanh per 2 chunks) -> square -> 4 q-matmuls accumulate; DVE
adds/subtracts q into dz halves; DMA out.  Matmuls are fp16 (1 cyc/col);
emission is software-pipelined one block deep so the activation engine
(the throughput bound at ~37.5k cols/core) never waits on the PE.  Host
scatters the result back by perm.
"""

import os
import numpy as np

B, D, H = 200003, 32, 128
NCORES = 8
P2 = 200000            # rows covered by pairs
RPC = P2 // NCORES     # 25000 rows per core
NCHUNK = 4
CH = RPC // NCHUNK     # 6250 rows per chunk
NPAIR = CH // 2        # 3125 real pairs per chunk (+1 pad pair)
WPB = 256              # pairs per block (full blocks)
NFULL = NPAIR // WPB   # 12 full blocks
WPT = NPAIR + 1 - NFULL * WPB   # 54 pairs in the tail block (incl 1 pad pair)
NBLK = NFULL + 1
NCOL = 2 * (NPAIR + 1)  # 6252 packed columns per chunk strip

_CACHE = {}
LAST_RESULTS = None    # BassKernelResults of the most recent run (for test.py)


def _blocks():
    """[(col_start, pairs_in_block), ...] for the 13 blocks."""
    out = [(2 * WPB * b, WPB) for b in range(NFULL)]
    out.append((2 * WPB * NFULL, WPT))
    return out


def build_program():
    """Build the single-core Bass/Tile program (same program runs SPMD on 8 cores)."""
    from contextlib import ExitStack
    import concourse.bacc as bacc
    import concourse.mybir as mybir
    import concourse.tile as tile

    dt = mybir.dt
    F32 = dt.float32
    F16 = dt.float16
    AF = mybir.ActivationFunctionType
    OP = mybir.AluOpType

    # One concatenated fp16 weight tensor [128, 1536]:
    #   w1rep[0:128] | pw1rep[128:256] | w2q[256:768] | pwtq[768:1280]
    #   | w1z[1280:1408] | pw1z[1408:1536]
    # w2q/pwtq are column-placed per chunk (vW2 at M-cols 32j of block j,
    # zeros elsewhere): matmul outputs must start at PSUM partition 0, so the
    # 4 chunk matmuls accumulate full-M into one [128,*] psum tile.
    # w1z/pw1z: chunk 3 is read from partition base 64 with K=64 and zeros in
    # rows 64:96 (partition base 96 is not encodable).
    nc = bacc.Bacc()
    X = nc.dram_tensor("x", [128, NCOL], F16, kind="ExternalInput")
    WC = nc.dram_tensor("wcat", [128, 1536], F16, kind="ExternalInput")
    BC = nc.dram_tensor("bias", [128, 2], F32, kind="ExternalInput")
    O = nc.dram_tensor("out", [128, NCOL], F32, kind="ExternalOutput")

    with tile.TileContext(nc) as tc, ExitStack() as ctx:
        wpool = ctx.enter_context(tc.tile_pool(name="wpool", bufs=1))
        xpool = ctx.enter_context(tc.tile_pool(name="xpool", bufs=3))
        dfpool = ctx.enter_context(tc.tile_pool(name="dfpool", bufs=2))
        utpool = ctx.enter_context(tc.tile_pool(name="utpool", bufs=2))
        vtpool = ctx.enter_context(tc.tile_pool(name="vtpool", bufs=2))
        sqpool = ctx.enter_context(tc.tile_pool(name="sqpool", bufs=2))
        opool = ctx.enter_context(tc.tile_pool(name="opool", bufs=3))
        qspool = ctx.enter_context(tc.tile_pool(name="qspool", bufs=2))
        hpool = ctx.enter_context(tc.tile_pool(name="hpool", bufs=2, space="PSUM"))
        papool = ctx.enter_context(tc.tile_pool(name="papool", bufs=1, space="PSUM"))
        dzpool = ctx.enter_context(tc.tile_pool(name="dzpool", bufs=1, space="PSUM"))
        qpool = ctx.enter_context(tc.tile_pool(name="qpool", bufs=1, space="PSUM"))

        wt = wpool.tile([128, 1536], F16)
        nc.sync.dma_start(wt[:], WC[:])
        bt = wpool.tile([128, 2], F32)
        nc.sync.dma_start(bt[:], BC[:])
        w1 = wt[:, 0:128]
        pw1 = wt[:, 128:256]
        w2q = wt[:, 256:768]
        pwtq = wt[:, 768:1280]
        w1z = wt[:, 1280:1408]
        pw1z = wt[:, 1408:1536]
        bh = bt[:, 0:1]
        pb1 = bt[:, 1:2]

        def g2(ap):
            """[128, 2*n] view of a [128, >=1024] tile as [128, 2, n] strided
            at 512 (used to read two bank-aligned chunk regions in one op)."""
            return ap.rearrange("p (g c) -> p g c", g=2)

        blocks = _blocks()
        pend = None  # (ut, sq, out col base, W, WP) of the previous block

        for b in range(NBLK + 1):
            if b < NBLK:
                c0, WP = blocks[b]
                W = 2 * WP
                xt = xpool.tile([128, 2 * WPB], F16)
                nc.sync.dma_start(xt[:, :W], X[:, c0 : c0 + W])
                df = dfpool.tile([128, WPB], F16)
                nc.vector.tensor_tensor(df[:, :WP], xt[:, 0:WP], xt[:, WP:W], OP.subtract)

                ut = utpool.tile([128, 8 * WPB], F16)
                vt = vtpool.tile([128, 4 * WPB], F16)
                # chunks 0,1 -> hps1 at bank-aligned offsets 0/512
                hps1 = hpool.tile([128, 1024], F32, tag="hps", name="hps")
                nc.tensor.matmul(hps1[:, 0:W], w1[0:32, :], xt[0:32, :W], start=True, stop=True)
                nc.tensor.matmul(hps1[:, 512 : 512 + W], w1[32:64, :], xt[32:64, :W], start=True, stop=True)
                if W == 512:
                    nc.scalar.activation(ut[:, 0:1024], hps1[:, 0:1024], AF.Tanh, bias=bh[:])
                else:
                    nc.scalar.activation(
                        g2(ut[:, 0 : 2 * W]), g2(hps1)[:, :, 0:W], AF.Tanh, bias=bh[:]
                    )
                # pair-net chunks 0,1 at bank-aligned offsets 0/512
                pa1 = papool.tile([128, 1024], F32, tag="pa", name="pa")
                nc.tensor.matmul(pa1[:, 0:WP], pw1[0:32, :], df[0:32, :WP], start=True, stop=True)
                nc.tensor.matmul(pa1[:, 512 : 512 + WP], pw1[32:64, :], df[32:64, :WP], start=True, stop=True)
                nc.scalar.activation(
                    g2(vt[:, 0 : 2 * WP]), g2(pa1)[:, :, 0:WP], AF.Tanh, bias=pb1[:]
                )
                # chunks 2,3
                hps2 = hpool.tile([128, 1024], F32, tag="hps", name="hps")
                nc.tensor.matmul(hps2[:, 0:W], w1[64:96, :], xt[64:96, :W], start=True, stop=True)
                nc.tensor.matmul(hps2[:, 512 : 512 + W], w1z[64:128, :], xt[64:128, :W], start=True, stop=True)
                if W == 512:
                    nc.scalar.activation(ut[:, 1024:2048], hps2[:, 0:1024], AF.Tanh, bias=bh[:])
                else:
                    nc.scalar.activation(
                        g2(ut[:, 2 * W : 4 * W]), g2(hps2)[:, :, 0:W], AF.Tanh, bias=bh[:]
                    )
                pa2 = papool.tile([128, 1024], F32, tag="pa", name="pa")
                nc.tensor.matmul(pa2[:, 0:WP], pw1[64:96, :], df[64:96, :WP], start=True, stop=True)
                nc.tensor.matmul(pa2[:, 512 : 512 + WP], pw1z[64:128, :], df[64:128, :WP], start=True, stop=True)
                nc.scalar.activation(
                    g2(vt[:, 2 * WP : 4 * WP]), g2(pa2)[:, :, 0:WP], AF.Tanh, bias=pb1[:]
                )
                sq = sqpool.tile([128, 4 * WPB], F16)
                nc.vector.tensor_mul(sq[:, : 4 * WP], vt[:, : 4 * WP], vt[:, : 4 * WP])

            if pend is not None:
                put, psq, pc0, pW, pWP = pend
                dz = dzpool.tile([128, 2 * WPB], F32)
                for j in range(4):
                    nc.tensor.matmul(
                        dz[:, :pW],
                        w2q[:, H * j : H * (j + 1)],
                        put[:, j * pW : (j + 1) * pW],
                        start=(j == 0),
                        stop=(j == 3),
                        skip_group_check=True,
                    )
                qp = qpool.tile([128, WPB], F32)
                for j in range(4):
                    nc.tensor.matmul(
                        qp[:, :pWP],
                        pwtq[:, H * j : H * (j + 1)],
                        psq[:, j * pWP : (j + 1) * pWP],
                        start=(j == 0),
                        stop=(j == 3),
                        skip_group_check=True,
                    )
                qs = qspool.tile([128, WPB], F32)
                nc.vector.tensor_copy(qs[:, :pWP], qp[:, :pWP])
                ot = opool.tile([128, 2 * WPB], F32)
                nc.vector.tensor_tensor(ot[:, 0:pWP], dz[:, 0:pWP], qs[:, :pWP], OP.add)
                nc.vector.tensor_tensor(ot[:, pWP:pW], dz[:, pWP:pW], qs[:, :pWP], OP.subtract)
                # output DMA on the (otherwise idle) gpsimd queue so input
                # and output transfers use different hardware DMA queues
                nc.gpsimd.dma_start(O[:, pc0 : pc0 + pW], ot[:, :pW])

            if b < NBLK:
                pend = (ut, sq, c0, W, WP)

    nc.compile()
    return nc


def _rowmaps():
    """col -> local row (0..6251) per chunk, and its inverse."""
    rowmap = np.empty(NCOL, dtype=np.int64)
    for b, (c0, WP) in enumerate(_blocks()):
        i = np.arange(WP)
        p = WPB * b + i
        rowmap[c0 + i] = 2 * p
        rowmap[c0 + WP + i] = 2 * p + 1
    invmap = np.empty_like(rowmap)
    invmap[rowmap] = np.arange(NCOL)
    return rowmap, invmap


_ROWMAP, _INVMAP = _rowmaps()


def _prep_weights(t, vW1, vb1, vW2, vb2, pW1, pb1, pW2):
    f32 = np.float32
    t = np.asarray(t, dtype=f32).reshape(-1)[0]
    vW1 = np.asarray(vW1, dtype=f32)
    w1rep = np.tile(np.ascontiguousarray(vW1[:32]), (4, 1))            # [128,128]
    biash = (np.asarray(vb1, f32) + t * vW1[32]).reshape(128, 1).astype(f32)
    vw2 = np.ascontiguousarray(np.asarray(vW2, f32))                   # [128,32]
    pW1 = np.asarray(pW1, f32)
    pw1rep = np.tile(pW1, (4, 1))                                      # [128,128]
    pb1c = np.asarray(pb1, f32).reshape(128, 1).copy()
    w2col = np.asarray(pW2, f32).reshape(128)
    pw1tw2 = np.ascontiguousarray((pW1 * w2col[None, :]).T)            # [128,32]
    z96 = np.zeros((96, 128), f32)
    w2q = np.zeros((128, 512), f32)
    pwtq = np.zeros((128, 512), f32)
    for j in range(4):
        w2q[:, 128 * j + 32 * j : 128 * j + 32 * j + 32] = vw2
        pwtq[:, 128 * j + 32 * j : 128 * j + 32 * j + 32] = pw1tw2
    w1z = np.vstack([z96, vW1[:32]])                                   # [128,128]
    pw1z = np.vstack([z96, pW1])                                       # [128,128]
    wcat = np.hstack([w1rep, pw1rep, w2q, pwtq, w1z, pw1z]).astype(np.float16)
    bias = np.hstack([biash, pb1c]).astype(f32)
    # constant part of g: c0[d] = sum_k pW1[d,k]*w2[k], in the fp16 weight
    # precision actually used on device
    c0base = pw1tw2.astype(np.float16).astype(f32).sum(axis=0)         # [32]
    return {"wcat": np.ascontiguousarray(wcat), "bias": np.ascontiguousarray(bias),
            "_c0base": c0base}


def _pack_core(zc):
    """[25000, 32] f32 -> [128, 6252] fp16 packed: partition 32*j+d holds dim d
    of chunk j; chunk-local columns follow the even/odd block layout."""
    zp = np.zeros((NCHUNK, NCOL, D), dtype=np.float32)
    zp[:, : CH, :] = zc.reshape(NCHUNK, CH, D)
    zp = zp[:, _ROWMAP, :]                                   # [4, 6252, 32]
    return zp.transpose(0, 2, 1).reshape(128, NCOL).astype(np.float16)


def _unpack_core(oc):
    """[128, 6252] packed f32 -> [25000, 32]."""
    o = oc.reshape(NCHUNK, D, NCOL).transpose(0, 2, 1)       # [4, 6252, 32]
    return np.ascontiguousarray(o[:, _INVMAP, :][:, :CH, :]).reshape(RPC, D)


def _host_triple(t, z3, vW1, vb1, vW2, vb2, pW1, pb1, pW2):
    """Exact float64 computation of the 3 leftover rows: dz_dt + triple forces."""
    f8 = np.float64
    z3 = z3.astype(f8)
    vW1 = np.asarray(vW1, f8)
    t = float(np.asarray(t).reshape(-1)[0])
    h3 = np.tanh(z3 @ vW1[:32] + t * vW1[32] + np.asarray(vb1, f8))
    dz3 = h3 @ np.asarray(vW2, f8) + np.asarray(vb2, f8)

    pW1 = np.asarray(pW1, f8)
    w2 = np.asarray(pW2, f8).reshape(128)
    d9 = (z3[:, None, :] - z3[None, :, :]).reshape(9, 32)
    u9 = np.tanh(d9 @ pW1 + np.asarray(pb1, f8))
    s9 = (1.0 - u9 * u9) * w2[None, :]
    g9 = s9 @ pW1.T                       # grad_phi rows
    f9 = (-g9).reshape(3, 3, 32)
    f9 = f9 * (1.0 - np.eye(3)[:, :, None])
    force3 = f9.sum(axis=1) * 2.0
    return (dz3 + force3).astype(np.float32)


def kernel(t, z, perm, vW1, vb1, vW2, vb2, pW1, pb1, pW2, pb2):
    from concourse.bass_utils import run_bass_kernel_spmd

    global LAST_RESULTS
    if "nc" not in _CACHE:
        _CACHE["nc"] = build_program()
    nc = _CACHE["nc"]

    z = np.asarray(z, np.float32)
    perm = np.asarray(perm)
    weights = _prep_weights(t, vW1, vb1, vW2, vb2, pW1, pb1, pW2)

    c0base = weights.pop("_c0base")
    zg = z[perm[:P2]]                       # [200000, 32] gathered pair rows
    in_maps = []
    for c in range(NCORES):
        im = {"x": _pack_core(zg[c * RPC : (c + 1) * RPC])}
        im.update(weights)
        in_maps.append(im)

    trace = bool(int(os.environ.get("KERNEL_TRACE", "0")))
    res = run_bass_kernel_spmd(nc, in_maps, list(range(NCORES)), trace=trace)
    LAST_RESULTS = res

    out = np.empty((B, 32), dtype=np.float32)
    og = np.concatenate([_unpack_core(res.results[c]["out"]) for c in range(NCORES)], axis=0)
    vb2f = np.asarray(vb2, np.float32)
    og[0::2] += (vb2f - c0base)[None, :]
    og[1::2] += (vb2f + c0base)[None, :]
    out[perm[:P2]] = og
    out[perm[P2:]] = _host_triple(t, z[perm[P2:]], vW1, vb1, vW2, vb2, pW1, pb1, pW2)
    return out


# revision 8
# speedup vs baseline: 1.3388x; 1.2131x over previous
"""Trainium2 Bass kernel for nn_ODEFunc_interaction (gnn_message_passing).

Math (see reference):
  dz_dt = tanh([z, t] @ vW1 + vb1) @ vW2 + vb2                    (v-net, all rows)
  for each pair (perm[2i], perm[2i+1]):
      d_i  = z[perm[2i]] - z[perm[2i+1]]
      g_i  = grad_phi(d_i) = pW1 @ (pW2[:,0] * (1 - tanh(d_i@pW1 + pb1)^2))
      out[perm[2i]]   = dz_dt[perm[2i]]   - g_i
      out[perm[2i+1]] = dz_dt[perm[2i+1]] + g_i
  last 3 rows (triple) + 53 ragged pairs/chunk handled on host (tiny).

Strategy: host gathers z[perm] so each of 8 cores owns 25000 rows (12500
pairs).  On-device layout is transposed+packed: X[128, 6144] fp16 where
partition 32*j+d holds dim d of row-chunk j (4 chunks x 3072 pairs).
Columns are grouped in superblocks of 512 pairs per chunk: 512 even
members then 512 odd members, so the pair-difference and the final +/-
combine run on contiguous ranges (DVE 4x fp16) and every tanh is one
contiguous 1024-col activation from bank-aligned PSUM (matmul PSUM
outputs must start at a bank boundary - mid-bank starts are fatal).
Per superblock: 8 h-matmuls -> 4 fused tanh -> 8 dz-matmuls; 4
pa-matmuls -> 2 fused tanh -> square -> 4 q-matmuls; DVE combines
+/-q into the dz halves; DMA out.  All matmuls fp16 (1 cyc/col).
Emission is software-pipelined one superblock deep so the activation
engine (the throughput bound, ~36.9k cols/core) never waits on the PE.
PSUM: one 3-slot rotating pool serves h/dz/qp tiles + a 1-slot pa pool
= exactly 8 banks.  Host scatters the result back by perm.
"""

import os
import numpy as np

B, D, H = 200003, 32, 128
NCORES = 8
P2 = 200000            # rows covered by pairs
RPC = P2 // NCORES     # 25000 rows per core
NCHUNK = 4
CH = RPC // NCHUNK     # 6250 rows per chunk
NPAIR = CH // 2        # 3125 pairs per chunk
WPB = 512              # pairs per superblock
NSB = NPAIR // WPB     # 6 full superblocks on device
DPAIR = NSB * WPB      # 3072 pairs per chunk on device
DROW = 2 * DPAIR       # 6144 device rows per chunk
NCOL = DROW            # 6144 packed columns per chunk strip

_CACHE = {}
LAST_RESULTS = None    # BassKernelResults of the most recent run (for test.py)


def build_program():
    """Build the single-core Bass/Tile program (same program runs SPMD on 8 cores)."""
    from contextlib import ExitStack
    import concourse.bacc as bacc
    import concourse.mybir as mybir
    import concourse.tile as tile

    dt = mybir.dt
    F32 = dt.float32
    F16 = dt.float16
    AF = mybir.ActivationFunctionType
    OP = mybir.AluOpType

    # One concatenated fp16 weight tensor [128, 1536]:
    #   w1rep[0:128] | pw1rep[128:256] | w1z[256:384] | pw1z[384:512]
    #   | w2q[512:1024] | pwtq[1024:1536]
    # The first 512 cols (layer-1 weights) are DMA'd separately so the first
    # h-matmul does not wait for the full weight transfer.
    # w2q/pwtq are column-placed per chunk (vW2 at M-cols 32j of block j,
    # zeros elsewhere): matmul outputs must start at PSUM partition 0, so the
    # 4 chunk matmuls accumulate full-M into one [128,*] psum tile.
    # w1z/pw1z: chunk 3 is read from partition base 64 with K=64 and zeros in
    # rows 64:96 (partition base 96 is not encodable).
    nc = bacc.Bacc()
    X = nc.dram_tensor("x", [128, NCOL], F16, kind="ExternalInput")
    WC = nc.dram_tensor("wcat", [128, 1536], F16, kind="ExternalInput")
    BC = nc.dram_tensor("bias", [128, 2], F32, kind="ExternalInput")
    O = nc.dram_tensor("out", [128, NCOL], F32, kind="ExternalOutput")

    with tile.TileContext(nc) as tc, ExitStack() as ctx:
        wpool = ctx.enter_context(tc.tile_pool(name="wpool", bufs=1))
        xpool = ctx.enter_context(tc.tile_pool(name="xpool", bufs=3))
        dfpool = ctx.enter_context(tc.tile_pool(name="dfpool", bufs=2))
        utpool = ctx.enter_context(tc.tile_pool(name="utpool", bufs=4))
        vtpool = ctx.enter_context(tc.tile_pool(name="vtpool", bufs=2))
        sqpool = ctx.enter_context(tc.tile_pool(name="sqpool", bufs=3))
        opool = ctx.enter_context(tc.tile_pool(name="opool", bufs=4))
        qspool = ctx.enter_context(tc.tile_pool(name="qspool", bufs=2))
        pspool = ctx.enter_context(tc.tile_pool(name="pspool", bufs=3, space="PSUM"))
        papool = ctx.enter_context(tc.tile_pool(name="papool", bufs=1, space="PSUM"))

        bt = wpool.tile([128, 2], F32)
        nc.sync.dma_start(bt[:], BC[:])
        wt = wpool.tile([128, 1536], F16)
        nc.sync.dma_start(wt[:, 0:512], WC[:, 0:512])      # layer-1 weights first
        w1 = wt[:, 0:128]
        pw1 = wt[:, 128:256]
        w1z = wt[:, 256:384]
        pw1z = wt[:, 384:512]
        w2q = wt[:, 512:1024]
        pwtq = wt[:, 1024:1536]
        bh = bt[:, 0:1]
        pb1 = bt[:, 1:2]

        pend = None  # (utA, utB, sq, col base) of the previous superblock

        for i in range(NSB + 1):
            if i < NSB:
                c0 = 2 * WPB * i
                xt = xpool.tile([128, 2 * WPB], F16)
                nc.sync.dma_start(xt[:], X[:, c0 : c0 + 2 * WPB])
                if i == 0:
                    # rest of the weights (layer-2) behind the first x block
                    nc.sync.dma_start(wt[:, 512:1536], WC[:, 512:1536])
                df = dfpool.tile([128, WPB], F16)
                nc.vector.tensor_tensor(df[:], xt[:, 0:WPB], xt[:, WPB:], OP.subtract)

                # h-block A = even members (cols 0:512), B = odd (512:1024)
                utA = utpool.tile([128, 2048], F16, tag="ut", name="ut")
                utB = utpool.tile([128, 2048], F16, tag="ut", name="ut")
                vt = vtpool.tile([128, 2048], F16)
                hA1 = pspool.tile([128, 1024], F32, tag="ps", name="ps")
                nc.tensor.matmul(hA1[:, 0:512], w1[0:32, :], xt[0:32, 0:512], start=True, stop=True)
                nc.tensor.matmul(hA1[:, 512:1024], w1[32:64, :], xt[32:64, 0:512], start=True, stop=True)
                nc.scalar.activation(utA[:, 0:1024], hA1[:], AF.Tanh, bias=bh[:])
                hA2 = pspool.tile([128, 1024], F32, tag="ps", name="ps")
                nc.tensor.matmul(hA2[:, 0:512], w1[64:96, :], xt[64:96, 0:512], start=True, stop=True)
                nc.tensor.matmul(hA2[:, 512:1024], w1z[64:128, :], xt[64:128, 0:512], start=True, stop=True)
                nc.scalar.activation(utA[:, 1024:2048], hA2[:], AF.Tanh, bias=bh[:])
                pa1 = papool.tile([128, 1024], F32, tag="pa", name="pa")
                nc.tensor.matmul(pa1[:, 0:512], pw1[0:32, :], df[0:32, :], start=True, stop=True)
                nc.tensor.matmul(pa1[:, 512:1024], pw1[32:64, :], df[32:64, :], start=True, stop=True)
                nc.scalar.activation(vt[:, 0:1024], pa1[:], AF.Tanh, bias=pb1[:])
                hB1 = pspool.tile([128, 1024], F32, tag="ps", name="ps")
                nc.tensor.matmul(hB1[:, 0:512], w1[0:32, :], xt[0:32, 512:1024], start=True, stop=True)
                nc.tensor.matmul(hB1[:, 512:1024], w1[32:64, :], xt[32:64, 512:1024], start=True, stop=True)
                nc.scalar.activation(utB[:, 0:1024], hB1[:], AF.Tanh, bias=bh[:])
                hB2 = pspool.tile([128, 1024], F32, tag="ps", name="ps")
                nc.tensor.matmul(hB2[:, 0:512], w1[64:96, :], xt[64:96, 512:1024], start=True, stop=True)
                nc.tensor.matmul(hB2[:, 512:1024], w1z[64:128, :], xt[64:128, 512:1024], start=True, stop=True)
                nc.scalar.activation(utB[:, 1024:2048], hB2[:], AF.Tanh, bias=bh[:])

            if pend is not None:
                putA, putB, psq, pc0 = pend
                # dz for both h-blocks of the previous superblock: one 2-bank
                # tile, halves bank-aligned; chunk-paired to reuse ldweights
                dz = pspool.tile([128, 1024], F32, tag="ps", name="ps")
                for j in range(4):
                    nc.tensor.matmul(
                        dz[:, 0:512],
                        w2q[:, H * j : H * (j + 1)],
                        putA[:, 512 * j : 512 * (j + 1)],
                        start=(j == 0),
                        stop=(j == 3),
                        skip_group_check=True,
                    )
                    nc.tensor.matmul(
                        dz[:, 512:1024],
                        w2q[:, H * j : H * (j + 1)],
                        putB[:, 512 * j : 512 * (j + 1)],
                        start=(j == 0),
                        stop=(j == 3),
                        skip_group_check=True,
                    )

            if i < NSB:
                pa2 = papool.tile([128, 1024], F32, tag="pa", name="pa")
                nc.tensor.matmul(pa2[:, 0:512], pw1[64:96, :], df[64:96, :], start=True, stop=True)
                nc.tensor.matmul(pa2[:, 512:1024], pw1z[64:128, :], df[64:128, :], start=True, stop=True)
                nc.scalar.activation(vt[:, 1024:2048], pa2[:], AF.Tanh, bias=pb1[:])
                sq = sqpool.tile([128, 2048], F16)
                nc.vector.tensor_mul(sq[:], vt[:], vt[:])

            if pend is not None:
                qp = pspool.tile([128, 1024], F32, tag="ps", name="ps")
                for j in range(4):
                    nc.tensor.matmul(
                        qp[:, 0:512],
                        pwtq[:, H * j : H * (j + 1)],
                        psq[:, 512 * j : 512 * (j + 1)],
                        start=(j == 0),
                        stop=(j == 3),
                        skip_group_check=True,
                    )
                qs = qspool.tile([128, WPB], F32)
                nc.vector.tensor_copy(qs[:], qp[:, 0:512])
                otA = opool.tile([128, WPB], F32, tag="ot", name="ot")
                nc.vector.tensor_tensor(otA[:], dz[:, 0:512], qs[:], OP.add)
                otB = opool.tile([128, WPB], F32, tag="ot", name="ot")
                nc.vector.tensor_tensor(otB[:], dz[:, 512:1024], qs[:], OP.subtract)
                # output DMA on the (otherwise idle) gpsimd queue so input
                # and output transfers use different hardware DMA queues
                nc.gpsimd.dma_start(O[:, pc0 : pc0 + WPB], otA[:])
                nc.gpsimd.dma_start(O[:, pc0 + WPB : pc0 + 2 * WPB], otB[:])

            if i < NSB:
                pend = (utA, utB, sq, 2 * WPB * i)

    nc.compile()
    return nc


def _rowmaps():
    """col -> local row (0..6143) per chunk, and its inverse."""
    rowmap = np.empty(NCOL, dtype=np.int64)
    for b in range(NSB):
        i = np.arange(WPB)
        p = WPB * b + i
        rowmap[2 * WPB * b + i] = 2 * p
        rowmap[2 * WPB * b + WPB + i] = 2 * p + 1
    invmap = np.empty_like(rowmap)
    invmap[rowmap] = np.arange(NCOL)
    return rowmap, invmap


_ROWMAP, _INVMAP = _rowmaps()


def _prep_weights(t, vW1, vb1, vW2, vb2, pW1, pb1, pW2):
    f32 = np.float32
    t = np.asarray(t, dtype=f32).reshape(-1)[0]
    vW1 = np.asarray(vW1, dtype=f32)
    w1rep = np.tile(np.ascontiguousarray(vW1[:32]), (4, 1))            # [128,128]
    biash = (np.asarray(vb1, f32) + t * vW1[32]).reshape(128, 1).astype(f32)
    vw2 = np.ascontiguousarray(np.asarray(vW2, f32))                   # [128,32]
    pW1 = np.asarray(pW1, f32)
    pw1rep = np.tile(pW1, (4, 1))                                      # [128,128]
    pb1c = np.asarray(pb1, f32).reshape(128, 1).copy()
    w2col = np.asarray(pW2, f32).reshape(128)
    pw1tw2 = np.ascontiguousarray((pW1 * w2col[None, :]).T)            # [128,32]
    z96 = np.zeros((96, 128), f32)
    w2q = np.zeros((128, 512), f32)
    pwtq = np.zeros((128, 512), f32)
    for j in range(4):
        w2q[:, 128 * j + 32 * j : 128 * j + 32 * j + 32] = vw2
        pwtq[:, 128 * j + 32 * j : 128 * j + 32 * j + 32] = pw1tw2
    w1z = np.vstack([z96, vW1[:32]])                                   # [128,128]
    pw1z = np.vstack([z96, pW1])                                       # [128,128]
    wcat = np.hstack([w1rep, pw1rep, w1z, pw1z, w2q, pwtq]).astype(np.float16)
    bias = np.hstack([biash, pb1c]).astype(f32)
    # constant part of g: c0[d] = sum_k pW1[d,k]*w2[k], in the fp16 weight
    # precision actually used on device
    c0base = pw1tw2.astype(np.float16).astype(f32).sum(axis=0)         # [32]
    return {"wcat": np.ascontiguousarray(wcat), "bias": np.ascontiguousarray(bias),
            "_c0base": c0base}


def _pack_core(zc):
    """[25000, 32] f32 -> [128, 6144] fp16 packed: partition 32*j+d holds dim d
    of chunk j; chunk-local columns follow the even/odd superblock layout."""
    zp = zc.reshape(NCHUNK, CH, D)[:, :DROW, :]              # [4, 6144, 32]
    zp = zp[:, _ROWMAP, :]
    return zp.transpose(0, 2, 1).reshape(128, NCOL).astype(np.float16)


def _unpack_core(oc):
    """[128, 6144] packed f32 -> [4, 6144, 32] in chunk-local row order."""
    o = oc.reshape(NCHUNK, D, NCOL).transpose(0, 2, 1)       # [4, 6144, 32]
    return o[:, _INVMAP, :]


def _host_vnet(t, zr, vW1, vb1, vW2, vb2):
    f8 = np.float64
    t = float(np.asarray(t).reshape(-1)[0])
    vW1 = np.asarray(vW1, f8)
    h = np.tanh(zr.astype(f8) @ vW1[:32] + t * vW1[32] + np.asarray(vb1, f8))
    return h @ np.asarray(vW2, f8) + np.asarray(vb2, f8)


def _host_pairs(t, zE, zO, vW1, vb1, vW2, vb2, pW1, pb1, pW2):
    """Exact v-net + pair force for leftover pairs: returns (outE, outO)."""
    f8 = np.float64
    pW1 = np.asarray(pW1, f8)
    w2 = np.asarray(pW2, f8).reshape(128)
    d = zE.astype(f8) - zO.astype(f8)
    u = np.tanh(d @ pW1 + np.asarray(pb1, f8))
    g = ((1.0 - u * u) * w2[None, :]) @ pW1.T               # grad_phi rows
    outE = _host_vnet(t, zE, vW1, vb1, vW2, vb2) - g
    outO = _host_vnet(t, zO, vW1, vb1, vW2, vb2) + g
    return outE.astype(np.float32), outO.astype(np.float32)


def _host_triple(t, z3, vW1, vb1, vW2, vb2, pW1, pb1, pW2):
    """Exact float64 computation of the 3 leftover rows: dz_dt + triple forces."""
    f8 = np.float64
    pW1 = np.asarray(pW1, f8)
    w2 = np.asarray(pW2, f8).reshape(128)
    z3 = z3.astype(f8)
    d9 = (z3[:, None, :] - z3[None, :, :]).reshape(9, 32)
    u9 = np.tanh(d9 @ pW1 + np.asarray(pb1, f8))
    s9 = (1.0 - u9 * u9) * w2[None, :]
    g9 = s9 @ pW1.T                       # grad_phi rows
    f9 = (-g9).reshape(3, 3, 32)
    f9 = f9 * (1.0 - np.eye(3)[:, :, None])
    force3 = f9.sum(axis=1) * 2.0
    return (_host_vnet(t, z3, vW1, vb1, vW2, vb2) + force3).astype(np.float32)


def kernel(t, z, perm, vW1, vb1, vW2, vb2, pW1, pb1, pW2, pb2):
    from concourse.bass_utils import run_bass_kernel_spmd

    global LAST_RESULTS
    if "nc" not in _CACHE:
        _CACHE["nc"] = build_program()
    nc = _CACHE["nc"]

    z = np.asarray(z, np.float32)
    perm = np.asarray(perm)
    weights = _prep_weights(t, vW1, vb1, vW2, vb2, pW1, pb1, pW2)

    c0base = weights.pop("_c0base")
    zg = z[perm[:P2]]                       # [200000, 32] gathered pair rows
    in_maps = []
    for c in range(NCORES):
        im = {"x": _pack_core(zg[c * RPC : (c + 1) * RPC])}
        im.update(weights)
        in_maps.append(im)

    trace = bool(int(os.environ.get("KERNEL_TRACE", "0")))
    res = run_bass_kernel_spmd(nc, in_maps, list(range(NCORES)), trace=trace)
    LAST_RESULTS = res

    out = np.empty((B, 32), dtype=np.float32)
    og = np.empty((P2, D), dtype=np.float32)
    vb2f = np.asarray(vb2, np.float32)
    dev_even = (vb2f - c0base)[None, :]
    dev_odd = (vb2f + c0base)[None, :]
    # leftover rows (local rows DROW..CH-1 of each chunk) computed on host
    lrow = np.arange(DROW, CH)
    lE = lrow[0::2]
    lO = lrow[1::2]
    for c in range(NCORES):
        od = _unpack_core(res.results[c]["out"])             # [4, 6144, 32]
        zc = zg[c * RPC : (c + 1) * RPC].reshape(NCHUNK, CH, D)
        for j in range(NCHUNK):
            base = c * RPC + j * CH
            blk = og[base : base + CH]
            blk[:DROW] = od[j]
            blk[:DROW:2] += dev_even
            blk[1:DROW:2] += dev_odd
            blk[lE], blk[lO] = _host_pairs(
                t, zc[j, lE], zc[j, lO], vW1, vb1, vW2, vb2, pW1, pb1, pW2
            )
    out[perm[:P2]] = og
    out[perm[P2:]] = _host_triple(t, z[perm[P2:]], vW1, vb1, vW2, vb2, pW1, pb1, pW2)
    return out


# revision 9
# speedup vs baseline: 1.3658x; 1.0201x over previous
"""Trainium2 Bass kernel for nn_ODEFunc_interaction (gnn_message_passing).

Math (see reference):
  dz_dt = tanh([z, t] @ vW1 + vb1) @ vW2 + vb2                    (v-net, all rows)
  for each pair (perm[2i], perm[2i+1]):
      d_i  = z[perm[2i]] - z[perm[2i+1]]
      g_i  = grad_phi(d_i) = pW1 @ (pW2[:,0] * (1 - tanh(d_i@pW1 + pb1)^2))
      out[perm[2i]]   = dz_dt[perm[2i]]   - g_i
      out[perm[2i+1]] = dz_dt[perm[2i+1]] + g_i
  last 3 rows (triple) + 53 ragged pairs/chunk handled on host (tiny).

Strategy: host gathers z[perm] so each of 8 cores owns 25000 rows (12500
pairs).  On-device layout is transposed+packed: X[128, 6144] fp16 where
partition 32*j+d holds dim d of row-chunk j (4 chunks x 3072 pairs).
Columns are grouped in superblocks of 512 pairs per chunk: 512 even
members then 512 odd members, so the pair-difference and the final +/-
combine run on contiguous ranges (DVE 4x fp16) and every tanh is one
contiguous 1024-col activation from bank-aligned PSUM (matmul PSUM
outputs must start at a bank boundary - mid-bank starts are fatal).
Per superblock: 8 h-matmuls -> 4 fused tanh -> 8 dz-matmuls; 4
pa-matmuls -> 2 fused tanh -> square -> 4 q-matmuls; DVE combines
+/-q into the dz halves; DMA out.  All matmuls fp16 (1 cyc/col).
Emission is software-pipelined one superblock deep so the activation
engine (the throughput bound, ~36.9k cols/core) never waits on the PE.
PSUM: one 3-slot rotating pool serves h/dz/qp tiles + a 1-slot pa pool
= exactly 8 banks.  Host scatters the result back by perm.
"""

import os
import numpy as np

B, D, H = 200003, 32, 128
NCORES = 8
P2 = 200000            # rows covered by pairs
RPC = P2 // NCORES     # 25000 rows per core
NCHUNK = 4
CH = RPC // NCHUNK     # 6250 rows per chunk
NPAIR = CH // 2        # 3125 pairs per chunk
WPB = 512              # pairs per superblock
NSB = NPAIR // WPB     # 6 full superblocks on device
DPAIR = NSB * WPB      # 3072 pairs per chunk on device
DROW = 2 * DPAIR       # 6144 device rows per chunk
NCOL = DROW            # 6144 packed columns per chunk strip

_CACHE = {}
LAST_RESULTS = None    # BassKernelResults of the most recent run (for test.py)


def build_program():
    """Build the single-core Bass/Tile program (same program runs SPMD on 8 cores)."""
    from contextlib import ExitStack
    import concourse.bacc as bacc
    import concourse.mybir as mybir
    import concourse.tile as tile

    dt = mybir.dt
    F32 = dt.float32
    F16 = dt.float16
    AF = mybir.ActivationFunctionType
    OP = mybir.AluOpType

    # One concatenated fp16 weight tensor [128, 1536]:
    #   w1rep[0:128] | w1z[128:256] | pw1rep[256:384] | pw1z[384:512]
    #   | w2q[512:1024] | pwtq[1024:1536]
    # The first 512 cols (layer-1 weights) are DMA'd separately so the first
    # h-matmul does not wait for the full weight transfer.
    # w2q/pwtq are column-placed per chunk (vW2 at M-cols 32j of block j,
    # zeros elsewhere): matmul outputs must start at PSUM partition 0, so the
    # 4 chunk matmuls accumulate full-M into one [128,*] psum tile.
    # w1z/pw1z: chunk 3 is read from partition base 64 with K=64 and zeros in
    # rows 64:96 (partition base 96 is not encodable).
    nc = bacc.Bacc()
    X = nc.dram_tensor("x", [128, NCOL], F16, kind="ExternalInput")
    WC = nc.dram_tensor("wcat", [128, 1536], F16, kind="ExternalInput")
    BC = nc.dram_tensor("bias", [128, 2], F32, kind="ExternalInput")
    O = nc.dram_tensor("out", [128, NCOL], F32, kind="ExternalOutput")

    with tile.TileContext(nc) as tc, ExitStack() as ctx:
        wpool = ctx.enter_context(tc.tile_pool(name="wpool", bufs=1))
        xpool = ctx.enter_context(tc.tile_pool(name="xpool", bufs=3))
        dfpool = ctx.enter_context(tc.tile_pool(name="dfpool", bufs=2))
        utpool = ctx.enter_context(tc.tile_pool(name="utpool", bufs=4))
        vtpool = ctx.enter_context(tc.tile_pool(name="vtpool", bufs=2))
        sqpool = ctx.enter_context(tc.tile_pool(name="sqpool", bufs=3))
        opool = ctx.enter_context(tc.tile_pool(name="opool", bufs=4))
        qspool = ctx.enter_context(tc.tile_pool(name="qspool", bufs=2))
        pspool = ctx.enter_context(tc.tile_pool(name="pspool", bufs=3, space="PSUM"))
        papool = ctx.enter_context(tc.tile_pool(name="papool", bufs=1, space="PSUM"))

        bt = wpool.tile([128, 2], F32)
        nc.sync.dma_start(bt[:], BC[:])
        wt = wpool.tile([128, 1536], F16)
        nc.sync.dma_start(wt[:, 0:256], WC[:, 0:256])      # h-net layer-1 weights first
        w1 = wt[:, 0:128]
        w1z = wt[:, 128:256]
        pw1 = wt[:, 256:384]
        pw1z = wt[:, 384:512]
        w2q = wt[:, 512:1024]
        pwtq = wt[:, 1024:1536]
        bh = bt[:, 0:1]
        pb1 = bt[:, 1:2]

        pend = None  # (utA, utB, sq, col base) of the previous superblock

        for i in range(NSB + 1):
            if i < NSB:
                c0 = 2 * WPB * i
                xt = xpool.tile([128, 2 * WPB], F16)
                if i == 0:
                    # first x block on the idle gpsimd queue, in parallel with
                    # the weight transfers on the sync queue
                    nc.gpsimd.dma_start(xt[:], X[:, c0 : c0 + 2 * WPB])
                    nc.sync.dma_start(wt[:, 256:512], WC[:, 256:512])
                    nc.sync.dma_start(wt[:, 512:1536], WC[:, 512:1536])
                else:
                    nc.sync.dma_start(xt[:], X[:, c0 : c0 + 2 * WPB])
                df = dfpool.tile([128, WPB], F16)
                nc.vector.tensor_tensor(df[:], xt[:, 0:WPB], xt[:, WPB:], OP.subtract)

                # h-block A = even members (cols 0:512), B = odd (512:1024)
                utA = utpool.tile([128, 2048], F16, tag="ut", name="ut")
                utB = utpool.tile([128, 2048], F16, tag="ut", name="ut")
                vt = vtpool.tile([128, 2048], F16)
                hA1 = pspool.tile([128, 1024], F32, tag="ps", name="ps")
                nc.tensor.matmul(hA1[:, 0:512], w1[0:32, :], xt[0:32, 0:512], start=True, stop=True)
                nc.tensor.matmul(hA1[:, 512:1024], w1[32:64, :], xt[32:64, 0:512], start=True, stop=True)
                nc.scalar.activation(utA[:, 0:1024], hA1[:], AF.Tanh, bias=bh[:])
                hA2 = pspool.tile([128, 1024], F32, tag="ps", name="ps")
                nc.tensor.matmul(hA2[:, 0:512], w1[64:96, :], xt[64:96, 0:512], start=True, stop=True)
                nc.tensor.matmul(hA2[:, 512:1024], w1z[64:128, :], xt[64:128, 0:512], start=True, stop=True)
                nc.scalar.activation(utA[:, 1024:2048], hA2[:], AF.Tanh, bias=bh[:])
                pa1 = papool.tile([128, 1024], F32, tag="pa", name="pa")
                nc.tensor.matmul(pa1[:, 0:512], pw1[0:32, :], df[0:32, :], start=True, stop=True)
                nc.tensor.matmul(pa1[:, 512:1024], pw1[32:64, :], df[32:64, :], start=True, stop=True)
                nc.scalar.activation(vt[:, 0:1024], pa1[:], AF.Tanh, bias=pb1[:])
                hB1 = pspool.tile([128, 1024], F32, tag="ps", name="ps")
                nc.tensor.matmul(hB1[:, 0:512], w1[0:32, :], xt[0:32, 512:1024], start=True, stop=True)
                nc.tensor.matmul(hB1[:, 512:1024], w1[32:64, :], xt[32:64, 512:1024], start=True, stop=True)
                nc.scalar.activation(utB[:, 0:1024], hB1[:], AF.Tanh, bias=bh[:])
                hB2 = pspool.tile([128, 1024], F32, tag="ps", name="ps")
                nc.tensor.matmul(hB2[:, 0:512], w1[64:96, :], xt[64:96, 512:1024], start=True, stop=True)
                nc.tensor.matmul(hB2[:, 512:1024], w1z[64:128, :], xt[64:128, 512:1024], start=True, stop=True)
                nc.scalar.activation(utB[:, 1024:2048], hB2[:], AF.Tanh, bias=bh[:])

            if pend is not None:
                putA, putB, psq, pc0 = pend
                # dz for both h-blocks of the previous superblock: one 2-bank
                # tile, halves bank-aligned; chunk-paired to reuse ldweights
                dz = pspool.tile([128, 1024], F32, tag="ps", name="ps")
                for j in range(4):
                    nc.tensor.matmul(
                        dz[:, 0:512],
                        w2q[:, H * j : H * (j + 1)],
                        putA[:, 512 * j : 512 * (j + 1)],
                        start=(j == 0),
                        stop=(j == 3),
                        skip_group_check=True,
                    )
                    nc.tensor.matmul(
                        dz[:, 512:1024],
                        w2q[:, H * j : H * (j + 1)],
                        putB[:, 512 * j : 512 * (j + 1)],
                        start=(j == 0),
                        stop=(j == 3),
                        skip_group_check=True,
                    )

            if i < NSB:
                pa2 = papool.tile([128, 1024], F32, tag="pa", name="pa")
                nc.tensor.matmul(pa2[:, 0:512], pw1[64:96, :], df[64:96, :], start=True, stop=True)
                nc.tensor.matmul(pa2[:, 512:1024], pw1z[64:128, :], df[64:128, :], start=True, stop=True)
                nc.scalar.activation(vt[:, 1024:2048], pa2[:], AF.Tanh, bias=pb1[:])
                sq = sqpool.tile([128, 2048], F16)
                nc.vector.tensor_mul(sq[:], vt[:], vt[:])

            if pend is not None:
                qp = pspool.tile([128, 1024], F32, tag="ps", name="ps")
                for j in range(4):
                    nc.tensor.matmul(
                        qp[:, 0:512],
                        pwtq[:, H * j : H * (j + 1)],
                        psq[:, 512 * j : 512 * (j + 1)],
                        start=(j == 0),
                        stop=(j == 3),
                        skip_group_check=True,
                    )
                qs = qspool.tile([128, WPB], F32)
                nc.vector.tensor_copy(qs[:], qp[:, 0:512])
                otA = opool.tile([128, WPB], F32, tag="ot", name="ot")
                nc.vector.tensor_tensor(otA[:], dz[:, 0:512], qs[:], OP.add)
                otB = opool.tile([128, WPB], F32, tag="ot", name="ot")
                nc.vector.tensor_tensor(otB[:], dz[:, 512:1024], qs[:], OP.subtract)
                # output DMA on the (otherwise idle) gpsimd queue so input
                # and output transfers use different hardware DMA queues
                nc.gpsimd.dma_start(O[:, pc0 : pc0 + WPB], otA[:])
                nc.gpsimd.dma_start(O[:, pc0 + WPB : pc0 + 2 * WPB], otB[:])

            if i < NSB:
                pend = (utA, utB, sq, 2 * WPB * i)

    nc.compile()
    return nc


def _rowmaps():
    """col -> local row (0..6143) per chunk, and its inverse."""
    rowmap = np.empty(NCOL, dtype=np.int64)
    for b in range(NSB):
        i = np.arange(WPB)
        p = WPB * b + i
        rowmap[2 * WPB * b + i] = 2 * p
        rowmap[2 * WPB * b + WPB + i] = 2 * p + 1
    invmap = np.empty_like(rowmap)
    invmap[rowmap] = np.arange(NCOL)
    return rowmap, invmap


_ROWMAP, _INVMAP = _rowmaps()


def _prep_weights(t, vW1, vb1, vW2, vb2, pW1, pb1, pW2):
    f32 = np.float32
    t = np.asarray(t, dtype=f32).reshape(-1)[0]
    vW1 = np.asarray(vW1, dtype=f32)
    w1rep = np.tile(np.ascontiguousarray(vW1[:32]), (4, 1))            # [128,128]
    biash = (np.asarray(vb1, f32) + t * vW1[32]).reshape(128, 1).astype(f32)
    vw2 = np.ascontiguousarray(np.asarray(vW2, f32))                   # [128,32]
    pW1 = np.asarray(pW1, f32)
    pw1rep = np.tile(pW1, (4, 1))                                      # [128,128]
    pb1c = np.asarray(pb1, f32).reshape(128, 1).copy()
    w2col = np.asarray(pW2, f32).reshape(128)
    pw1tw2 = np.ascontiguousarray((pW1 * w2col[None, :]).T)            # [128,32]
    z96 = np.zeros((96, 128), f32)
    w2q = np.zeros((128, 512), f32)
    pwtq = np.zeros((128, 512), f32)
    for j in range(4):
        w2q[:, 128 * j + 32 * j : 128 * j + 32 * j + 32] = vw2
        pwtq[:, 128 * j + 32 * j : 128 * j + 32 * j + 32] = pw1tw2
    w1z = np.vstack([z96, vW1[:32]])                                   # [128,128]
    pw1z = np.vstack([z96, pW1])                                       # [128,128]
    wcat = np.hstack([w1rep, w1z, pw1rep, pw1z, w2q, pwtq]).astype(np.float16)
    bias = np.hstack([biash, pb1c]).astype(f32)
    # constant part of g: c0[d] = sum_k pW1[d,k]*w2[k], in the fp16 weight
    # precision actually used on device
    c0base = pw1tw2.astype(np.float16).astype(f32).sum(axis=0)         # [32]
    return {"wcat": np.ascontiguousarray(wcat), "bias": np.ascontiguousarray(bias),
            "_c0base": c0base}


def _pack_core(zc):
    """[25000, 32] f32 -> [128, 6144] fp16 packed: partition 32*j+d holds dim d
    of chunk j; chunk-local columns follow the even/odd superblock layout."""
    zp = zc.reshape(NCHUNK, CH, D)[:, :DROW, :]              # [4, 6144, 32]
    zp = zp[:, _ROWMAP, :]
    return zp.transpose(0, 2, 1).reshape(128, NCOL).astype(np.float16)


def _unpack_core(oc):
    """[128, 6144] packed f32 -> [4, 6144, 32] in chunk-local row order."""
    o = oc.reshape(NCHUNK, D, NCOL).transpose(0, 2, 1)       # [4, 6144, 32]
    return o[:, _INVMAP, :]


def _host_vnet(t, zr, vW1, vb1, vW2, vb2):
    f8 = np.float64
    t = float(np.asarray(t).reshape(-1)[0])
    vW1 = np.asarray(vW1, f8)
    h = np.tanh(zr.astype(f8) @ vW1[:32] + t * vW1[32] + np.asarray(vb1, f8))
    return h @ np.asarray(vW2, f8) + np.asarray(vb2, f8)


def _host_pairs(t, zE, zO, vW1, vb1, vW2, vb2, pW1, pb1, pW2):
    """Exact v-net + pair force for leftover pairs: returns (outE, outO)."""
    f8 = np.float64
    pW1 = np.asarray(pW1, f8)
    w2 = np.asarray(pW2, f8).reshape(128)
    d = zE.astype(f8) - zO.astype(f8)
    u = np.tanh(d @ pW1 + np.asarray(pb1, f8))
    g = ((1.0 - u * u) * w2[None, :]) @ pW1.T               # grad_phi rows
    outE = _host_vnet(t, zE, vW1, vb1, vW2, vb2) - g
    outO = _host_vnet(t, zO, vW1, vb1, vW2, vb2) + g
    return outE.astype(np.float32), outO.astype(np.float32)


def _host_triple(t, z3, vW1, vb1, vW2, vb2, pW1, pb1, pW2):
    """Exact float64 computation of the 3 leftover rows: dz_dt + triple forces."""
    f8 = np.float64
    pW1 = np.asarray(pW1, f8)
    w2 = np.asarray(pW2, f8).reshape(128)
    z3 = z3.astype(f8)
    d9 = (z3[:, None, :] - z3[None, :, :]).reshape(9, 32)
    u9 = np.tanh(d9 @ pW1 + np.asarray(pb1, f8))
    s9 = (1.0 - u9 * u9) * w2[None, :]
    g9 = s9 @ pW1.T                       # grad_phi rows
    f9 = (-g9).reshape(3, 3, 32)
    f9 = f9 * (1.0 - np.eye(3)[:, :, None])
    force3 = f9.sum(axis=1) * 2.0
    return (_host_vnet(t, z3, vW1, vb1, vW2, vb2) + force3).astype(np.float32)


def kernel(t, z, perm, vW1, vb1, vW2, vb2, pW1, pb1, pW2, pb2):
    from concourse.bass_utils import run_bass_kernel_spmd

    global LAST_RESULTS
    if "nc" not in _CACHE:
        _CACHE["nc"] = build_program()
    nc = _CACHE["nc"]

    z = np.asarray(z, np.float32)
    perm = np.asarray(perm)
    weights = _prep_weights(t, vW1, vb1, vW2, vb2, pW1, pb1, pW2)

    c0base = weights.pop("_c0base")
    zg = z[perm[:P2]]                       # [200000, 32] gathered pair rows
    in_maps = []
    for c in range(NCORES):
        im = {"x": _pack_core(zg[c * RPC : (c + 1) * RPC])}
        im.update(weights)
        in_maps.append(im)

    trace = bool(int(os.environ.get("KERNEL_TRACE", "0")))
    res = run_bass_kernel_spmd(nc, in_maps, list(range(NCORES)), trace=trace)
    LAST_RESULTS = res

    out = np.empty((B, 32), dtype=np.float32)
    og = np.empty((P2, D), dtype=np.float32)
    vb2f = np.asarray(vb2, np.float32)
    dev_even = (vb2f - c0base)[None, :]
    dev_odd = (vb2f + c0base)[None, :]
    # leftover rows (local rows DROW..CH-1 of each chunk) computed on host
    lrow = np.arange(DROW, CH)
    lE = lrow[0::2]
    lO = lrow[1::2]
    for c in range(NCORES):
        od = _unpack_core(res.results[c]["out"])             # [4, 6144, 32]
        zc = zg[c * RPC : (c + 1) * RPC].reshape(NCHUNK, CH, D)
        for j in range(NCHUNK):
            base = c * RPC + j * CH
            blk = og[base : base + CH]
            blk[:DROW] = od[j]
            blk[:DROW:2] += dev_even
            blk[1:DROW:2] += dev_odd
            blk[lE], blk[lO] = _host_pairs(
                t, zc[j, lE], zc[j, lO], vW1, vb1, vW2, vb2, pW1, pb1, pW2
            )
    out[perm[:P2]] = og
    out[perm[P2:]] = _host_triple(t, z[perm[P2:]], vW1, vb1, vW2, vb2, pW1, pb1, pW2)
    return out
